# revision 25
# baseline (speedup 1.0000x reference)
"""Trainium2 Bass kernel for nn_MultiHeadAttention (B=4, S=2048, D=1024, H=16).

Sharding: 8 cores = 4 batches x 2 query-halves. Each core computes full K/V
projections for its batch (keys are permuted so the core's own queries come
first), attention for its 1024 queries over all 2048 keys, and the output
projection for its query half. No collectives needed.

Per-core dataflow (bf16 matmuls, fp32 PSUM accumulation, all tensors SBUF
resident — no DRAM spills):
  x [2048,1024] bf16 --PE transpose--> xT [D,S]
  qT = (x @ Wq)^T [D,1024] and v_aug = [x @ Wv | ones] computed up front;
  kT d-tiles are projected per head-pair, interleaved into the attention
  loop so the PE fills the gaps of the ACT(exp)-paced inner loop.
  Per head pair p, per q-span of 512:
    scoresT[k,q] tiles via row-paired (tile_position) K=64 matmuls
    exp on ACT (scale=1/8 folded in), flash-style, no max subtraction
    outT[65,q] accumulated in PSUM via v_aug=[v_h | ones] stationary
    normalize by row 64 (denominator via reciprocal_approx_fast +
    K=1 fp32 PE broadcast), add bv, assemble oT [D, Sq]
  y = oT^T @ Wo + bo  -> [1024, 1024] fp32
"""

import os
import numpy as np
import ml_dtypes
from contextlib import ExitStack

import concourse.bass as bass
from concourse import bacc
import concourse.mybir as mybir
import concourse.tile as tile
from concourse.bass_utils import run_bass_kernel_spmd
from concourse.masks import make_identity

F32 = mybir.dt.float32
BF16 = mybir.dt.bfloat16
AF = mybir.ActivationFunctionType
NPBF16 = ml_dtypes.bfloat16

P = 128

N_CORES = 8
B_FULL, S_FULL, D_FULL = 4, 2048, 1024
H_FULL, DH = 16, 64


def build_mha_nc(S=2048, Sq=1024, D=1024, H=16, scale=None):
    """Build the per-core Bass program. Returns nc."""
    assert D % P == 0 and S % P == 0 and Sq % P == 0 and H % 2 == 0
    ND = D // P            # d-tiles
    NS = S // P            # s-chunks / k-tiles
    NPAIR = H // 2
    W65 = DH + 1           # augmented head width (v | ones)
    QSP = min(512, Sq)     # q span
    NQS = Sq // QSP
    KSP = min(512, S)      # span for kT projection
    NKS = S // KSP
    CSP = min(512, D)      # col span for v / out projections
    NCS = D // CSP
    HPS = CSP // DH        # heads per col-span in v projection
    if scale is None:
        scale = DH ** -0.5

    nc = bacc.Bacc(target_bir_lowering=False, debug=False)

    x = nc.dram_tensor("x", [S, D], BF16, kind="ExternalInput").ap()
    W = {n: nc.dram_tensor(n, [D, D], BF16, kind="ExternalInput").ap()
         for n in ("Wq", "Wk", "Wv", "Wo")}
    bias = {n: nc.dram_tensor(n, [D], F32, kind="ExternalInput").ap()
            for n in ("bq", "bk", "bv", "bo")}
    ones_d = nc.dram_tensor("cst_ones", [P, P], BF16, kind="ExternalInput").ap()
    y = nc.dram_tensor("y", [Sq, D], F32, kind="ExternalOutput").ap()

    with tile.TileContext(nc) as tc, ExitStack() as top:
        top.enter_context(nc.allow_low_precision(
            reason="bf16 activations/weights with fp32 psum accumulation"))
        const = top.enter_context(tc.tile_pool(name="const", bufs=1))
        big = top.enter_context(tc.tile_pool(name="big", bufs=1))
        wp = top.enter_context(tc.tile_pool(name="wp", bufs=2))
        kpool = top.enter_context(tc.tile_pool(name="kpool", bufs=3))
        ppsL = top.enter_context(tc.tile_pool(name="ppsL", bufs=1, space="PSUM"))

        ident = const.tile([P, P], BF16)
        make_identity(nc, ident)
        # bf16 ones row: K=1 stationary broadcasting the softmax denominator
        ones_t = const.tile([1, DH], BF16)
        nc.vector.memset(ones_t, 1.0)

        # per-partition bias layouts: b_sb[p, j] = b[j*128 + p]
        bq_sb = const.tile([P, ND], F32)
        nc.gpsimd.dma_start(out=bq_sb, in_=bias["bq"].rearrange("(j p) -> p j", p=P))
        bk_sb = const.tile([P, ND], F32)
        nc.gpsimd.dma_start(out=bk_sb, in_=bias["bk"].rearrange("(j p) -> p j", p=P))
        # bv split by head parity within a d-tile (used at base partition 0)
        bv_even = const.tile([DH, ND], F32)
        nc.gpsimd.dma_start(
            out=bv_even,
            in_=bias["bv"].rearrange("(j q p) -> q p j", p=DH, q=2)[0],
        )
        bv_odd = const.tile([DH, ND], F32)
        nc.gpsimd.dma_start(
            out=bv_odd,
            in_=bias["bv"].rearrange("(j q p) -> q p j", p=DH, q=2)[1],
        )
        # bo broadcast across partitions (0-stride DRAM read)
        bo_bc = const.tile([P, D], F32)
        nc.gpsimd.dma_start(
            out=bo_bc,
            in_=bias["bo"].unsqueeze(0).partition_broadcast(P).squeeze(1),
        )

        oT = big.tile([P, ND, Sq], BF16)
        xT = big.tile([P, ND, S], BF16)
        qTs = big.tile([P, ND, Sq], BF16)
        v_sb = big.tile([P, NS, H * W65], BF16)

        # ---- Phase T: PE-transpose x into xT ----
        with tc.tile_pool(name="xchunk", bufs=3) as xpool, \
             tc.tile_pool(name="tps", bufs=2, space="PSUM") as tpsum, \
             tc.tile_pool(name="ppsE", bufs=4, space="PSUM") as ppsE:
            for i in range(NS):
                xc = xpool.tile([P, D], BF16, tag="xc")
                nc.sync.dma_start(out=xc, in_=x[i * P:(i + 1) * P, :])
                for j in range(ND):
                    tp = tpsum.tile([P, P], BF16, tag="tp")
                    nc.tensor.transpose(tp, xc[:, j * P:(j + 1) * P], ident)
                    nc.vector.tensor_copy(xT[:, j, i * P:(i + 1) * P], tp)

            # ---- qT / v projections: helpers; early part emits only what
            # pair 0 needs, the rest interleaves into the attention loop ----
            Wq_sb = wp.tile([P, ND, D], BF16, tag="w")
            nc.sync.dma_start(out=Wq_sb, in_=W["Wq"].rearrange("(j p) c -> p j c", p=P))
            Wv_sb = wp.tile([P, ND, D], BF16, tag="wv", bufs=1)
            nc.sync.dma_start(out=Wv_sb, in_=W["Wv"].rearrange("(j p) c -> p j c", p=P))
            v3 = v_sb.rearrange("p i (h w) -> p i h w", w=W65)

            def qT_proj(dc, pool):
                for sp in range(NQS):
                    ps = pool.tile([P, QSP], F32, tag="pp", name=f"qps_{dc}_{sp}")
                    for j in range(ND):
                        nc.tensor.matmul(
                            ps,
                            Wq_sb[:, j, dc * P:(dc + 1) * P],
                            xT[:, j, sp * QSP:(sp + 1) * QSP],
                            start=(j == 0), stop=(j == ND - 1),
                        )
                    nc.vector.tensor_scalar_add(
                        qTs[:, dc, sp * QSP:(sp + 1) * QSP], ps, bq_sb[:, dc:dc + 1])

            def v_proj(i, sp, pool):
                if sp == 0:
                    nc.sync.dma_start(out=v3[:, i, :, DH:DH + 1],
                                      in_=ones_d[:, 0:H].unsqueeze(2))
                ps = pool.tile([P, CSP], F32, tag="pp", name=f"vps_{i}_{sp}")
                for j in range(ND):
                    nc.tensor.matmul(
                        ps,
                        xT[:, j, i * P:(i + 1) * P],
                        Wv_sb[:, j, sp * CSP:(sp + 1) * CSP],
                        start=(j == 0), stop=(j == ND - 1),
                    )
                nc.vector.tensor_copy(
                    v3[:, i, sp * HPS:(sp + 1) * HPS, 0:DH],
                    ps.rearrange("p (h w) -> p h w", w=DH),
                )

            Wk_sb = wp.tile([P, ND, D], BF16, tag="w")
            nc.sync.dma_start(out=Wk_sb, in_=W["Wk"].rearrange("(j p) c -> p j c", p=P))

            def kT_proj(p):
                kp = kpool.tile([P, S], BF16, tag="kp", name=f"kp_{p}")
                for sp in range(NKS):
                    ps = ppsL.tile([P, KSP], F32, tag="pp")
                    for j in range(ND):
                        nc.tensor.matmul(
                            ps,
                            Wk_sb[:, j, p * P:(p + 1) * P],
                            xT[:, j, sp * KSP:(sp + 1) * KSP],
                            start=(j == 0), stop=(j == ND - 1),
                        )
                    nc.vector.tensor_scalar_add(
                        kp[:, sp * KSP:(sp + 1) * KSP], ps, bk_sb[:, p:p + 1])
                return kp

            for dc in range(2):
                qT_proj(dc, ppsE)
            kps = [kT_proj(0), kT_proj(1)]
            for i in range(NS):
                v_proj(i, 0, ppsE)

        # ---- Attention (kT projection of pair p+2 interleaved) ----
        with tc.tile_pool(name="exp", bufs=4) as exq, \
             tc.tile_pool(name="eps", bufs=4) as eps, \
             tc.tile_pool(name="scps", bufs=2, space="PSUM") as scps, \
             tc.tile_pool(name="ops", bufs=2, space="PSUM") as opsum:
            for p in range(NPAIR):
                kp = kps[p]
                for sp in range(NQS):
                    qsl = slice(sp * QSP, (sp + 1) * QSP)
                    o_even = opsum.tile([W65, QSP], F32, tag="op")
                    o_odd = opsum.tile([W65, QSP], F32, tag="op")
                    for kt in range(NS):
                        sc = scps.tile([P, 2 * QSP], F32, tag="sc")
                        nc.tensor.matmul(
                            sc[:, 0:QSP],
                            kp[0:DH, kt * P:(kt + 1) * P],
                            qTs[0:DH, p, qsl],
                            start=True, stop=True,
                        )
                        nc.tensor.matmul(
                            sc[:, QSP:2 * QSP],
                            kp[DH:P, kt * P:(kt + 1) * P],
                            qTs[DH:P, p, qsl],
                            start=True, stop=True,
                        )
                        ex = exq.tile([P, 2 * QSP], BF16, tag="ex")
                        nc.scalar.activation(ex, sc, AF.Exp, scale=float(scale))
                        nc.tensor.matmul(
                            o_even,
                            v3[:, kt, 2 * p, :],
                            ex[:, 0:QSP],
                            start=(kt == 0), stop=(kt == NS - 1),
                        )
                        nc.tensor.matmul(
                            o_odd,
                            v3[:, kt, 2 * p + 1, :],
                            ex[:, QSP:2 * QSP],
                            start=(kt == 0), stop=(kt == NS - 1),
                        )
                    # epilogue: normalize + bias, assemble oT
                    for par, ops in ((0, o_even), (1, o_odd)):
                        den0 = eps.tile([1, QSP], F32, tag="den0")
                        nc.vector.tensor_copy(den0, ops[DH:W65, :])
                        rc0 = eps.tile([1, QSP], F32, tag="rc0")
                        nc.vector.reciprocal_approx_fast(rc0, den0)
                        rc0b = eps.tile([1, QSP], BF16, tag="rc0b")
                        nc.vector.tensor_copy(rc0b, rc0)
                        rb_ps = ppsL.tile([DH, QSP], F32, tag="rb")
                        nc.tensor.matmul(
                            rb_ps, ones_t, rc0b,
                            start=True, stop=True,
                        )
                        rb = eps.tile([DH, QSP], F32, tag="rb_sb")
                        nc.vector.tensor_copy(rb, rb_ps)
                        bv_sb = bv_even if par == 0 else bv_odd
                        if par == 0:
                            dst = oT[0:DH, p, qsl]
                            nc.vector.tensor_mul(dst, ops[0:DH, :], rb)
                            nc.vector.tensor_scalar_add(dst, dst, bv_sb[:, p:p + 1])
                        else:
                            on = eps.tile([DH, QSP], BF16, tag="on")
                            nc.vector.tensor_mul(on, ops[0:DH, :], rb)
                            nc.vector.tensor_scalar_add(on, on, bv_sb[:, p:p + 1])
                            nc.sync.dma_start(out=oT[DH:P, p, qsl], in_=on)
                if p + 2 < NPAIR:
                    qT_proj(p + 2, ppsL)
                    kps.append(kT_proj(p + 2))
                if p < 4 and NCS > 1:
                    for i in range(p * NS // 4, (p + 1) * NS // 4):
                        v_proj(i, 1, ppsL)

        # ---- Output projection ----
        Wo_sb = wp.tile([P, ND, D], BF16, tag="w")
        nc.sync.dma_start(out=Wo_sb, in_=W["Wo"].rearrange("(j p) c -> p j c", p=P))
        with tc.tile_pool(name="ystg", bufs=4) as ystg, \
             tc.tile_pool(name="yps", bufs=4, space="PSUM") as yps:
            for sc_i in range(Sq // P):
                for sp in range(NCS):
                    ps = yps.tile([P, CSP], F32, tag="yp")
                    for j in range(ND):
                        nc.tensor.matmul(
                            ps,
                            oT[:, j, sc_i * P:(sc_i + 1) * P],
                            Wo_sb[:, j, sp * CSP:(sp + 1) * CSP],
                            start=(j == 0), stop=(j == ND - 1),
                        )
                    ysb = ystg.tile([P, CSP], F32, tag="ysb")
                    nc.vector.tensor_add(ysb, ps, bo_bc[:, sp * CSP:(sp + 1) * CSP])
                    nc.sync.dma_start(
                        out=y[sc_i * P:(sc_i + 1) * P, sp * CSP:(sp + 1) * CSP],
                        in_=ysb,
                    )

    nc.compile()
    return nc


_NC = None


def _get_nc():
    global _NC
    if _NC is None:
        _NC = build_mha_nc(S=S_FULL, Sq=S_FULL // 2, D=D_FULL, H=H_FULL)
    return _NC


def shard_inputs(inputs):
    x = np.asarray(inputs["x"], dtype=np.float32).astype(NPBF16)
    wnames = ("Wq", "Wk", "Wv", "Wo")
    bnames = ("bq", "bk", "bv", "bo")
    shared = {n: np.ascontiguousarray(
        np.asarray(inputs[n], dtype=np.float32).astype(NPBF16)) for n in wnames}
    shared.update({n: np.ascontiguousarray(np.asarray(inputs[n], dtype=np.float32))
                   for n in bnames})
    shared["cst_ones"] = np.ones((P, P), dtype=NPBF16)
    half = S_FULL // 2
    maps = []
    for c in range(N_CORES):
        b, h = divmod(c, 2)
        xb = x[b]
        xp = np.concatenate([xb[h * half:(h + 1) * half],
                             xb[(1 - h) * half:(2 - h) * half]], axis=0)
        m = dict(shared)
        m["x"] = np.ascontiguousarray(xp)
        maps.append(m)
    return maps


def run(inputs, trace=False):
    nc = _get_nc()
    maps = shard_inputs(inputs)
    res = run_bass_kernel_spmd(nc, maps, list(range(N_CORES)), trace=trace)
    half = S_FULL // 2
    y = np.empty((B_FULL, S_FULL, D_FULL), dtype=np.float32)
    for c in range(N_CORES):
        b, h = divmod(c, 2)
        y[b, h * half:(h + 1) * half] = res.results[c]["y"]
    return y, res


def kernel(**inputs):
    y, _ = run(inputs, trace=False)
    return y
